# revision 7
# baseline (speedup 1.0000x reference)
"""NLI classifier (embedding -> shared-weight LSTM x2 -> MLP) on 8 trn2 cores.

Strategy (pure data parallel, transposed-state layout):
  - 1024 sequences (512 s1 + 512 s2) sharded 128/core as ONE merged chain:
    core k owns rows [64k, 64k+64) of both s1 and s2; batch = [s1 rows | s2
    rows] = 128 instances. One instruction covers all 128 (cost ~ free dim).
  - State kept TRANSPOSED: hT[p, k, b] = h[b, 128k+p] so the cell update
    directly produces the next step's matmul rhs - no PE transposes.
  - Recurrent gates^T computed per gate tile: ps[c, half, b] += sum_p
    WhhT[p, k, m, c] * hT[p, k, b]; weights stationary (lhsT), 16 MMs of
    [128x128]x[128x128] per step. xg (host-gathered token projections,
    bias folded) injected via one id128 matmul per gate (N=256).
  - 4 PSUM gate tiles (f, i, g, o) in separate banks, double-buffered
    (8 banks): each sigmoid fires as soon as its gate's MMs finish while
    the PE writes the next gate's bank. g-gate uses Tanh directly.
  - DVE cell update: c = f*c + i*g (c fp32), h = o*tanh(c) written straight
    into the transposed hT state tile.
  - MLP head on device from the final hT tile; output [3, 64] f32 per core.
"""

import numpy as np
import ml_dtypes

import concourse.bass as bass
import concourse.bacc as bacc
import concourse.mybir as mybir
import concourse.tile as tile
from concourse.bass_utils import run_bass_kernel_spmd

BF16 = ml_dtypes.bfloat16

VOCAB = 50000
E = 128
H = 256
G = 4 * H  # 1024
B = 512
T = 256
N_CORES = 8
PB = B // N_CORES   # 64 rows per core per sentence
MB = 2 * PB         # 128 merged instances per core
CH = 16             # timesteps per DMA chunk

FP32 = mybir.dt.float32
BF = mybir.dt.bfloat16
AF = mybir.ActivationFunctionType
ALU = mybir.AluOpType

# gate order along the m axis (m-tile = 128 gate rows): f, i, g, o
GATES = ("f", "i", "g", "o")
GFUNC = {"f": AF.Sigmoid, "i": AF.Sigmoid, "g": AF.Tanh, "o": AF.Sigmoid}

_CACHE = {}


def _build():
    nc = bacc.Bacc("TRN2", target_bir_lowering=False, debug=False,
                   num_devices=N_CORES)

    # xg[p, t, m, b] = table2[tok[b, t], 128m + p]; per-partition contiguous
    xg_in = nc.dram_tensor("xg", [128, T, 8, MB], BF, kind="ExternalInput").ap()
    # whhT[p, k, m, c] = Whh_perm[128m + c, 128k + p]
    whhT_in = nc.dram_tensor("whhT", [128, 2, 8, 128], BF,
                             kind="ExternalInput").ap()
    id128_in = nc.dram_tensor("id128", [128, 128], BF, kind="ExternalInput").ap()
    whidT_in = nc.dram_tensor("whidT", [128, 4, H], BF, kind="ExternalInput").ap()
    bhid_in = nc.dram_tensor("bhid", [1, H], FP32, kind="ExternalInput").ap()
    woutT_in = nc.dram_tensor("woutT", [128, 2, 3], BF, kind="ExternalInput").ap()
    bout_in = nc.dram_tensor("bout", [1, 3], FP32, kind="ExternalInput").ap()
    out_dram = nc.dram_tensor("out", [3, PB], FP32, kind="ExternalOutput").ap()

    with tile.TileContext(nc) as tc:
        with (
            tc.tile_pool(name="const", bufs=1) as cpool,
            tc.tile_pool(name="state", bufs=1) as spool,
            tc.tile_pool(name="xg", bufs=2) as xgpool,
            tc.tile_pool(name="work", bufs=2) as wpool,
            tc.tile_pool(name="gpsum", bufs=1, space="PSUM") as gpsum,
        ):
            # ---- constants (id128 first: inject t=0 needs only it + xg[0]) ----
            id128 = cpool.tile([128, 128], BF, tag="id128")
            nc.sync.dma_start(out=id128[:], in_=id128_in[:, :])
            whhT = cpool.tile([128, 2, 8, 128], BF, tag="whhT")
            whidT = cpool.tile([128, 4, H], BF, tag="whidT")
            bhid = cpool.tile([1, H], FP32, tag="bhid")
            woutT = cpool.tile([128, 2, 3], BF, tag="woutT")
            bout = cpool.tile([1, 3], FP32, tag="bout")
            ones = cpool.tile([1, PB], FP32, tag="ones")
            nc.gpsimd.memset(ones[:], 1.0)

            # ---- persistent state ----
            # hT[p, k, b]: h for hidden unit 128k+p of instance b
            hT = [spool.tile([128, 2 * MB], BF, tag=f"hT{j}", name=f"hT{j}")
                  for j in range(2)]
            c_st = spool.tile([128, 2 * MB], BF, tag="c", name="cst")

            def emit_step(t, xg_j):
                first = t == 0
                h_prev = hT[t % 2]
                h_new = hT[(t + 1) % 2]
                ps = {g: gpsum.tile([128, 2 * MB], FP32, tag=f"ps_{g}{t % 2}",
                                    name=f"ps{g}")
                      for g in GATES}
                # inject xg (one id-matmul per gate, N=256)
                for gi, g in enumerate(GATES):
                    nc.tensor.matmul(ps[g][:, :], lhsT=id128[:],
                                     rhs=xg_j[:, 2 * gi:2 * gi + 2, :],
                                     start=True, stop=first,
                                     skip_group_check=True)
                sig = {}
                for gi, g in enumerate(GATES):
                    if not first:
                        for half in range(2):
                            m = 2 * gi + half
                            for k in range(2):
                                nc.tensor.matmul(
                                    ps[g][:, half * MB:(half + 1) * MB],
                                    lhsT=whhT[:, k, m, :],
                                    rhs=h_prev[:, k * MB:(k + 1) * MB],
                                    start=False, stop=(k == 1),
                                    skip_group_check=True)
                    s = wpool.tile([128, 2 * MB], BF, tag=f"sig_{g}",
                                   name=f"sig{g}")
                    nc.scalar.activation(s[:], ps[g][:], GFUNC[g])
                    sig[g] = s
                    if g == "f" and not first:
                        nc.vector.tensor_tensor(c_st[:], sig["f"][:], c_st[:],
                                                op=ALU.mult)
                    if g == "g":
                        if first:
                            nc.vector.tensor_tensor(c_st[:], sig["i"][:],
                                                    sig["g"][:], op=ALU.mult)
                        else:
                            u = wpool.tile([128, 2 * MB], BF, tag="u", name="uu")
                            nc.vector.tensor_tensor(u[:], sig["i"][:],
                                                    sig["g"][:], op=ALU.mult)
                            nc.vector.tensor_tensor(c_st[:], c_st[:], u[:],
                                                    op=ALU.add)
                tc_ = wpool.tile([128, 2 * MB], BF, tag="tc", name="tct")
                nc.scalar.activation(tc_[:], c_st[:], AF.Tanh)
                nc.vector.tensor_tensor(h_new[:], sig["o"][:], tc_[:],
                                        op=ALU.mult)

            n_chunks = T // CH
            for chunk in range(n_chunks):
                xt = xgpool.tile([128, CH, 8, MB], BF, tag="xg", name="xgt")
                if chunk == 0:
                    # small first piece so compute starts ASAP; whhT (needed
                    # from t=1) and head consts stream in behind it
                    pieces = [(0, 2), (2, 8), (8, CH)]
                else:
                    pieces = [(0, CH // 2), (CH // 2, CH)]
                for lo, hi in pieces:
                    nc.sync.dma_start(
                        out=xt[:, lo:hi, :, :],
                        in_=xg_in[:, chunk * CH + lo:chunk * CH + hi, :, :])
                    if chunk == 0 and lo == 0:
                        nc.sync.dma_start(out=whhT[:], in_=whhT_in[:, :, :, :])
                if chunk == 0:
                    nc.sync.dma_start(out=whidT[:], in_=whidT_in[:, :, :])
                    nc.sync.dma_start(out=bhid[:], in_=bhid_in[:, :])
                    nc.sync.dma_start(out=woutT[:], in_=woutT_in[:, :, :])
                    nc.sync.dma_start(out=bout[:], in_=bout_in[:, :])
                for j in range(CH):
                    emit_step(chunk * CH + j, xt[:, j, :, :])

            # ---- MLP head ----
            hfin = hT[T % 2]
            # cat = [h1 | h2]: k-tiles [h1 k0, h1 k1, h2 k0, h2 k1]
            catT = [hfin[:, 0:PB], hfin[:, MB:MB + PB],
                    hfin[:, PB:MB], hfin[:, MB + PB:2 * MB]]
            hidT = wpool.tile([128, 2, PB], BF, tag="hidT")
            for m in range(2):
                hp = gpsum.tile([128, 2 * MB], FP32, tag="ps_f0", name="hp")
                for k4 in range(4):
                    nc.tensor.matmul(hp[:, 0:PB],
                                     lhsT=whidT[:, k4, m * 128:(m + 1) * 128],
                                     rhs=catT[k4], start=(k4 == 0), stop=False,
                                     skip_group_check=True)
                nc.tensor.matmul(hp[:, 0:PB],
                                 lhsT=bhid[:, m * 128:(m + 1) * 128],
                                 rhs=ones[:], start=False, stop=True,
                                 skip_group_check=True)
                nc.scalar.activation(hidT[:, m, :], hp[:, 0:PB], AF.Relu)
            lp = gpsum.tile([128, 2 * MB], FP32, tag="ps_i0", name="lp")
            for m in range(2):
                nc.tensor.matmul(lp[0:3, 0:PB], lhsT=woutT[:, m, :],
                                 rhs=hidT[:, m, :],
                                 start=(m == 0), stop=False,
                                 skip_group_check=True)
            nc.tensor.matmul(lp[0:3, 0:PB], lhsT=bout[:], rhs=ones[:],
                             start=False, stop=True, skip_group_check=True)
            logits = wpool.tile([3, PB], FP32, tag="logits")
            nc.vector.tensor_copy(logits[:], lp[0:3, 0:PB])
            nc.sync.dma_start(out=out_dram[:, :], in_=logits[:])

    nc.compile()
    return nc


LAST_RESULT = None


def kernel(s1, s2, emb, w_ih, w_hh, b_ih, b_hh, w_hid, b_hid, w_out, b_out,
           _trace=False):
    global LAST_RESULT
    s1 = np.asarray(s1)
    s2 = np.asarray(s2)
    emb = np.asarray(emb, np.float32)
    w_ih = np.asarray(w_ih, np.float32)
    w_hh = np.asarray(w_hh, np.float32)
    b_ih = np.asarray(b_ih, np.float32)
    b_hh = np.asarray(b_hh, np.float32)
    w_hid = np.asarray(w_hid, np.float32)
    b_hid = np.asarray(b_hid, np.float32)
    w_out = np.asarray(w_out, np.float32)
    b_out = np.asarray(b_out, np.float32)

    # gate permutation [i|f|g|o] -> [f|i|g|o] (m-tile order)
    perm = np.concatenate([np.arange(H, 2 * H), np.arange(0, H),
                           np.arange(2 * H, 4 * H)])
    # host precompute: projected + biased gate table (bias folded)
    table2 = (emb @ w_ih[perm].T + (b_ih + b_hh)[perm]).astype(BF16)  # [V, G]
    # whhT[p, k, m, c] = Whh_perm[128m + c, 128k + p]
    whhT = np.ascontiguousarray(
        w_hh[perm].reshape(8, 128, 2, 128).transpose(3, 2, 0, 1)).astype(BF16)

    if "nc" not in _CACHE:
        _CACHE["nc"] = _build()
    nc = _CACHE["nc"]

    id128 = np.eye(128, dtype=BF16)
    # whidT[p, k4, c] = w_hid[c, 128 k4 + p]
    whidT = np.ascontiguousarray(
        w_hid.T.reshape(4, 128, H).transpose(1, 0, 2)).astype(BF16)
    # woutT[p, m, j] = w_out[j, 128 m + p]
    woutT = np.ascontiguousarray(
        w_out.T.reshape(2, 128, 3).transpose(1, 0, 2)).astype(BF16)

    in_maps = []
    for k in range(N_CORES):
        sl = slice(k * PB, (k + 1) * PB)
        tok = np.concatenate([s1[sl], s2[sl]], axis=0)       # [MB, T]
        gath = table2[tok]                                   # [MB, T, G] bf16
        # xg[p, t, m, b] = gath[b, t, 128m + p]
        xg = np.ascontiguousarray(
            gath.reshape(MB, T, 8, 128).transpose(3, 1, 2, 0))
        in_maps.append({
            "xg": xg,
            "whhT": whhT,
            "id128": id128,
            "whidT": whidT,
            "bhid": b_hid.reshape(1, H).astype(np.float32),
            "woutT": woutT,
            "bout": b_out.reshape(1, 3).astype(np.float32),
        })

    res = run_bass_kernel_spmd(nc, in_maps, list(range(N_CORES)), trace=_trace)
    LAST_RESULT = res
    out = np.empty((B, 3), np.float32)
    for k in range(N_CORES):
        out[k * PB:(k + 1) * PB] = res.results[k]["out"].T
    return out
